# revision 19
# baseline (speedup 1.0000x reference)
"""
CIN (Compressed Interaction Network) kernel for Trainium2, 8 NeuronCores.

Problem (hardcoded):
  x: [4096, 32, 64] fp32; w0: [128, 1024]; b0: [128]; w1: [128, 2048]; b1: [128]
  out: [4096, 192] = concat(relu(y0)[:, 64:], relu(y1)).sum(d)

Sharding: data parallel over batch, 512 samples/core, tokens t=(b,d), T=32768.

Key structure (per core, software-pipelined over 2048-token pairs):
  - L0 via polarization: x_h*x_f = ((x_h+x_f)^2 - x_h^2 - x_f^2)/2 ->
    496 upper-triangle sum-channels + 32 squares = 528 channels (vs 1024).
    Built on the PE as K=32 two-hot matmuls (4-way tile_position
    concurrency over the 4 x-copies); ScalarE evacuates with func=Square;
    contraction uses host-folded weights.
  - L1 z-channels are PERMUTED to (f, h) order so the broadcast operand is
    built from x (a pure input): xe_g[p, t] = x[2g + p//64, t] is
    DMA-expanded from HBM (13 groups) or PE-built via one-hot selects
    (3 groups, ScalarE evac). The hidden side is one 2x-tiled SBUF copy
    per pair (hd2[p] = hidden[p % 64]).
  - Block B interleaves: L1 muls+contract of pair B-1 with the PE s-builds
    of pair B (one build group per 2 contract groups), so the PE stream
    never has a >2us bubble (keeps HAM at K=8/8). L0 contract of pair B
    runs at block tail, producing hd2(B) before block B+1's muls need it.
  - d-sums via log2 trees of strided 2x-mode VectorE adds (not
    tensor_reduce, which is capped at 1x).
"""

import sys

import numpy as np
import ml_dtypes

sys.path.insert(0, "/opt/trn_rl_repo")

B_FULL = 4096
N_CORES = 8
BS = B_FULL // N_CORES  # 512
F = 32
D = 64
T = BS * D
PAIR = 2048
O = 128
H1 = 64
G1 = 16
CH0 = 528
NG0 = 5

DMA_G = list(range(14))   # groups delivered by plain DMA from host-expanded xe1h
PE_G = [14, 15]           # groups built on the PE from one-hot selects
GPS_MULS = (14, 15)       # z1 mul groups run on GpSimd instead of VectorE

_CACHE = {}


def _build_nc(BS=BS):
    import concourse.bass as bass
    import concourse.tile as tile
    from concourse import bacc, mybir

    T = BS * D
    NPAIR = T // PAIR
    SPP = PAIR // D

    bf16 = mybir.dt.bfloat16
    f32 = mybir.dt.float32
    Relu = mybir.ActivationFunctionType.Relu
    Square = mybir.ActivationFunctionType.Square
    Copy = mybir.ActivationFunctionType.Copy

    nc = bacc.Bacc(None, target_bir_lowering=False)

    xt = nc.dram_tensor("xt", [NPAIR, 128, PAIR], bf16, kind="ExternalInput")
    # host-expanded broadcast rows, packed per pair as 2 bulk transfers of
    # 7 groups each: [NPAIR, half, 128, 7*PAIR]
    xe1h = nc.dram_tensor(
        "xe1h", [NPAIR, 2, 128, 7 * PAIR], bf16, kind="ExternalInput"
    )
    selx = nc.dram_tensor("selx", [128, len(PE_G), 128], bf16, kind="ExternalInput")
    a0 = nc.dram_tensor("a0", [128, NG0, 128], bf16, kind="ExternalInput")
    w0f = nc.dram_tensor("w0f", [128, NG0, O], bf16, kind="ExternalInput")
    w1g = nc.dram_tensor("w1g", [128, G1, O], bf16, kind="ExternalInput")
    b0 = nc.dram_tensor("b0", [O, 1], f32, kind="ExternalInput")
    b1 = nc.dram_tensor("b1", [O, 1], f32, kind="ExternalInput")
    out0 = nc.dram_tensor("out0", [O - H1, BS], f32, kind="ExternalOutput")
    out1 = nc.dram_tensor("out1", [O, BS], f32, kind="ExternalOutput")

    with tile.TileContext(nc) as tc:
        with (
            tc.tile_pool(name="singles", bufs=1) as singles,
            tc.tile_pool(name="xrp", bufs=2) as xrp,
            tc.tile_pool(name="s0p", bufs=6) as s0p,
            tc.tile_pool(name="xep", bufs=4) as xep,
            tc.tile_pool(name="xcp", bufs=2) as xcp,
            tc.tile_pool(name="hd2p", bufs=2) as hd2p,
            tc.tile_pool(name="y0sbp", bufs=2) as y0sbp,
            tc.tile_pool(name="y1sbp", bufs=2) as y1sbp,
            tc.tile_pool(name="trp", bufs=2) as trp,
            tc.tile_pool(name="bcps", bufs=2, space="PSUM") as bcps,
            tc.tile_pool(name="yqp", bufs=4, space="PSUM") as yqp,
        ):
            a0s = singles.tile([128, NG0, 128], bf16)
            selxs = singles.tile([128, len(PE_G), 128], bf16)
            w0fs = singles.tile([128, NG0, O], bf16)
            w1s = singles.tile([128, G1, O], bf16)
            b0s = singles.tile([O, 1], f32)
            b1s = singles.tile([O, 1], f32)
            oacc0 = singles.tile([O, BS], f32)
            oacc1 = singles.tile([O, BS], f32)

            nc.gpsimd.dma_start(out=a0s[:], in_=a0[:])
            nc.gpsimd.dma_start(out=selxs[:], in_=selx[:])
            nc.gpsimd.dma_start(out=w0fs[:], in_=w0f[:])
            nc.gpsimd.dma_start(out=w1s[:], in_=w1g[:])
            nc.gpsimd.dma_start(out=b0s[:], in_=b0[:])
            nc.gpsimd.dma_start(out=b1s[:], in_=b1[:])

            st = {}

            def s1_load(P):
                """xt DMA, one block ahead (small, feeds the PE builds).
                HWDGE (sync) so it bypasses the SWDGE bulk-xe FIFO."""
                xr = xrp.tile([128, PAIR], bf16, name=f"xr{P}", tag="xr")
                nc.sync.dma_start(out=xr[:], in_=xt[P, :, :])
                st.setdefault(P, {})["xr"] = xr

            def prefetch_xe(P):
                """xe_g[p, t] = x[2g + p//64, t]; groups 0-13 via 2 bulk SWDGE
                reads of the host-expanded xe1h (7 groups each), 2 PE-built."""
                xeA = xep.tile([128, 7, PAIR], bf16, name=f"xeA{P}", tag="xe")
                xeB = xep.tile([128, 7, PAIR], bf16, name=f"xeB{P}", tag="xe")
                xeC = xcp.tile([128, 2, PAIR], bf16, name=f"xeC{P}", tag="xc")
                nc.gpsimd.dma_start(
                    out=xeA[:], in_=xe1h[P, 0, :, :].rearrange("p (g t) -> p g t", g=7)
                )
                nc.gpsimd.dma_start(
                    out=xeB[:], in_=xe1h[P, 1, :, :].rearrange("p (g t) -> p g t", g=7)
                )
                st[P]["xes"] = (xeA, xeB, xeC)
                st[P]["s0"] = []

            def xeg_ap(P, g):
                xeA, xeB, xeC = st[P]["xes"]
                if g < 7:
                    return xeA[:, g, :]
                if g < 14:
                    return xeB[:, g - 7, :]
                return xeC[:, g - 14, :]

            def build_xe_pe(P, k):
                """PE-built broadcast for PE_G[k]: one-hot selects of x rows.
                Evac on ScalarE (VectorE is the mul pole)."""
                g = PE_G[k]
                xr = st[P]["xr"]
                psA = bcps.tile([128, 2, 512], f32, name=f"xpsA{P}_{k}", tag="bc")
                psB = bcps.tile([128, 2, 512], f32, name=f"xpsB{P}_{k}", tag="bc")
                for c in range(4):
                    ps = psA if c < 2 else psB
                    nc.tensor.matmul(
                        ps[:, c % 2, :],
                        selxs[32 * c : 32 * c + 32, k, :],
                        xr[32 * c : 32 * c + 32, 512 * c : 512 * (c + 1)],
                        start=True, stop=True, tile_position=(32 * c, 0),
                    )
                xeg = xeg_ap(P, g)
                for half, ps in ((0, psA), (1, psB)):
                    dst = xeg[:, 1024 * half : 1024 * (half + 1)].rearrange(
                        "p (j c) -> p j c", j=2
                    )
                    nc.scalar.activation(dst, ps[:], Copy)

            def l0_half(P, half):
                """L0 contract for tokens [1024*half, 1024*(half+1)) into a
                2-bank bcps tile; ScalarE Relu evac into y0sb; hd2 half-copy.
                Runs mid-block so hd2(P) is ready before the tail muls."""
                s0g = st[P]["s0"]
                ps = bcps.tile([128, 2, 512], f32, name=f"l0q{P}_{half}", tag="bc")
                for G in range(NG0):
                    for q in range(2):
                        nc.tensor.matmul(
                            ps[:, q, :], w0fs[:, G, :],
                            s0g[G][:, 1024 * half + 512 * q : 1024 * half + 512 * (q + 1)],
                            start=(G == 0), stop=(G == NG0 - 1),
                        )
                if half == 0:
                    y0sb = y0sbp.tile([128, PAIR], bf16, name=f"y0sb{P}", tag="y0sb")
                    st[P]["y0sb"] = y0sb
                    hd2 = hd2p.tile([128, PAIR], bf16, name=f"hd2_{P}", tag="hd2")
                    st[P]["hd2"] = hd2
                y0sb, hd2 = st[P]["y0sb"], st[P]["hd2"]
                dst = y0sb[:, 1024 * half : 1024 * (half + 1)].rearrange(
                    "p (j c) -> p j c", j=2
                )
                nc.scalar.activation(dst, ps[:], Relu, bias=b0s[:])
                sl = slice(1024 * half, 1024 * (half + 1))
                nc.sync.dma_start(out=hd2[0:64, sl], in_=y0sb[0:H1, sl])
                nc.sync.dma_start(out=hd2[64:128, sl], in_=y0sb[0:H1, sl])

            def s1_build(P, G):
                xr = st[P]["xr"]
                psA = bcps.tile([128, 2, 512], f32, name=f"psA{P}_{G}", tag="bc")
                psB = bcps.tile([128, 2, 512], f32, name=f"psB{P}_{G}", tag="bc")
                for c in range(4):
                    ps = psA if c < 2 else psB
                    nc.tensor.matmul(
                        ps[:, c % 2, :],
                        a0s[32 * c : 32 * c + 32, G, :],
                        xr[32 * c : 32 * c + 32, 512 * c : 512 * (c + 1)],
                        start=True, stop=True, tile_position=(32 * c, 0),
                    )
                sg = s0p.tile([128, PAIR], bf16, name=f"s0_{P}_{G}", tag="s0")
                for half, ps in ((0, psA), (1, psB)):
                    dst = sg[:, 1024 * half : 1024 * (half + 1)].rearrange(
                        "p (j c) -> p j c", j=2
                    )
                    nc.scalar.activation(dst, ps[:], Square)
                st[P]["s0"].append(sg)

            tr_n = [0]

            def dtree(dst, src):
                """sum over d (64, innermost) via log2 tree of 2x-mode adds.
                src: [p, 32, 64] view; dst: [p, 32]."""
                p = src.shape[0]
                tr_n[0] += 1
                tr = trp.tile([128, 32, 32], bf16, name=f"tr{tr_n[0]}", tag="tr")
                t = tr[0:p]
                nc.vector.tensor_add(t[:, :, 0:32], src[:, :, 0:32], src[:, :, 32:64])
                nc.vector.tensor_add(t[:, :, 0:16], t[:, :, 0:16], t[:, :, 16:32])
                nc.vector.tensor_add(t[:, :, 0:8], t[:, :, 0:8], t[:, :, 8:16])
                nc.vector.tensor_add(t[:, :, 0:4], t[:, :, 0:4], t[:, :, 4:8])
                nc.vector.tensor_add(t[:, :, 0:2], t[:, :, 0:2], t[:, :, 2:4])
                nc.vector.tensor_add(dst, t[:, :, 0], t[:, :, 1])

            def red0(P):
                """out0 d-sum for pair P (y0sb ready since prior block tail)."""
                dtree(
                    oacc0[H1:O, P * SPP : (P + 1) * SPP],
                    st[P]["y0sb"][H1:O, :].rearrange("p (b d) -> p b d", d=D),
                )

            def s3_alloc(P):
                y1sb = y1sbp.tile([128, PAIR], bf16, name=f"y1sb{P}", tag="y1sb")
                yq = [
                    yqp.tile([128, 512], f32, name=f"y1q{P}_{q}", tag="yq")
                    for q in range(4)
                ]
                st[P]["y1sb"] = y1sb
                st[P]["yq"] = yq

            def s3_mul(P, g):
                xeg, hd2 = xeg_ap(P, g), st[P]["hd2"]
                eng = nc.gpsimd if g in GPS_MULS else nc.vector
                eng.tensor_mul(xeg, xeg, hd2[:])

            def s3_contract(P, g):
                xeg, yq = xeg_ap(P, g), st[P]["yq"]
                for q in range(4):
                    nc.tensor.matmul(
                        yq[q][:], w1s[:, g, :],
                        xeg[:, 512 * q : 512 * (q + 1)],
                        start=(g == 0), stop=(g == G1 - 1),
                    )

            def s3_fini(P):
                y1sb, yq = st[P]["y1sb"], st[P]["yq"]
                for q in range(4):
                    nc.scalar.activation(
                        y1sb[:, 512 * q : 512 * (q + 1)], yq[q][:], Relu, bias=b1s[:]
                    )
                dtree(
                    oacc1[:, P * SPP : (P + 1) * SPP],
                    y1sb[:].rearrange("p (b d) -> p b d", d=D),
                )
                del st[P]

            for B in range(NPAIR + 1):
                # pc: pair being multiplied + L1-contracted (muls 0,1 already
                # emitted at tail of block B-1); pn: pair being built (s0,
                # xe_pe, L0 halves, hd2) + prefetched.
                pc, pn = B - 1, B
                have_c = pc >= 0
                have_n = pn < NPAIR
                if B == 0:
                    s1_load(0)
                if have_n:
                    st.setdefault(pn, {})
                    if pn + 1 < NPAIR:
                        s1_load(pn + 1)
                    prefetch_xe(pn)
                if have_c:
                    s3_alloc(pc)
                    for g in GPS_MULS:
                        s3_mul(pc, g)  # gpsimd, long-running: issue early
                # build/L0 ops of pair pn by g-slot:
                #   g=0..4 s1_build, g=5,6 xe_pe, g=9,11 L0 halves
                for g in range(G1):
                    if have_c:
                        s3_contract(pc, g)
                        if g + 2 < G1 and (g + 2) not in GPS_MULS:
                            s3_mul(pc, g + 2)
                        if g == 6:
                            red0(pc)  # vector has ~1.5us slack by here
                    if have_n:
                        if g < NG0:
                            s1_build(pn, g)
                        elif g in (5, 6):
                            build_xe_pe(pn, g - NG0)
                        elif g in (9, 11):
                            l0_half(pn, (g - 9) // 2)
                if have_n:
                    # tail muls of pair pn (hd2(pn) ready mid-block)
                    s3_mul(pn, 0)
                    s3_mul(pn, 1)
                if have_c:
                    s3_fini(pc)

            nc.gpsimd.dma_start(out=out0[:], in_=oacc0[H1:O, :])
            nc.gpsimd.dma_start(out=out1[:], in_=oacc1[:])

    nc.finalize()
    return nc


def _get_nc():
    if "nc" not in _CACHE:
        _CACHE["nc"] = _build_nc()
    return _CACHE["nc"]


def _l0_pairs():
    return [(h, f) for h in range(F) for f in range(h + 1, F)]


def make_l0(w0_np):
    """A0 build matrix [32, 640] and folded weights [640, 128] (zero-padded)."""
    pairs = _l0_pairs()
    A0 = np.zeros((F, NG0 * 128), np.float32)
    w0fold = np.zeros((NG0 * 128, O), np.float32)
    for k, (h, f) in enumerate(pairs):
        A0[h, k] = 1.0
        A0[f, k] = 1.0
        w0fold[k] = (w0_np[:, h * F + f] + w0_np[:, f * F + h]) / 2
    for h in range(F):
        k = 496 + h
        A0[h, k] = 1.0
        c = w0_np[:, h * F + h].copy()
        for f in range(F):
            if f != h:
                c -= 0.5 * (w0_np[:, h * F + f] + w0_np[:, f * F + h])
        w0fold[k] = c
    return A0, w0fold


def make_xe1h(x_core_bf):
    """Host-expanded broadcast rows, packed per pair as 2 bulk transfers of
    7 groups each: [NPAIR, 2, 128, 7*PAIR] (partition-major, groups inner)."""
    row = np.empty(len(DMA_G) * 128, np.int64)
    p = np.arange(128)
    for i, g in enumerate(DMA_G):
        row[128 * i : 128 * (i + 1)] = 2 * g + p // 64
    full = x_core_bf[row]  # [14*128, T]
    npair = full.shape[1] // PAIR
    full = full.reshape(2, 7, 128, npair, PAIR)
    # -> [npair, half, 128, 7, PAIR] -> [npair, 2, 128, 7*PAIR]
    out = np.ascontiguousarray(full.transpose(3, 0, 2, 1, 4))
    return out.reshape(npair, 2, 128, 7 * PAIR)


def make_selx():
    sel = np.zeros((128, len(PE_G), 128), np.float32)
    for k, g in enumerate(PE_G):
        for s in range(4):
            for m in range(128):
                sel[32 * s + 2 * g + m // 64, k, m] = 1.0
    return sel


def make_w1g(w1_np):
    """Permuted L1 weights [128, 16, 128]: channel (g, p) -> h*F + f with
    f = 2g + p//64, h = p%64."""
    w1t = np.ascontiguousarray(np.asarray(w1_np, dtype=np.float32).T)  # [2048, O]
    p = np.arange(128)
    out = np.empty((128, G1, O), np.float32)
    for g in range(G1):
        c = (p % 64) * F + 2 * g + p // 64
        out[:, g, :] = w1t[c]
    return out


def kernel(cin_inputs, w0, b0, w1, b1, _trace=False):
    from concourse.bass_utils import run_bass_kernel_spmd

    x = np.asarray(cin_inputs, dtype=np.float32)
    assert x.shape == (B_FULL, F, D)
    bf = ml_dtypes.bfloat16
    xt_all = np.ascontiguousarray(
        x.reshape(N_CORES, BS, F, D).transpose(0, 2, 1, 3)
    ).astype(bf).reshape(N_CORES, F, BS * D)
    xt_all = np.ascontiguousarray(np.tile(xt_all, (1, 4, 1)))  # [NC, 128, T]
    NP_ = T // PAIR
    xt_pairs = np.ascontiguousarray(
        xt_all.reshape(N_CORES, 128, NP_, PAIR).transpose(0, 2, 1, 3)
    )  # [NC, NPAIR, 128, PAIR]

    w0_np = np.asarray(w0, dtype=np.float32)
    A0, w0fold = make_l0(w0_np)
    a0c = np.ascontiguousarray(
        np.tile(A0.reshape(F, NG0, 128), (4, 1, 1))
    ).astype(bf)
    w0fc = np.ascontiguousarray(
        w0fold.reshape(NG0, 128, O).transpose(1, 0, 2)
    ).astype(bf)
    w1gc = np.ascontiguousarray(make_w1g(w1)).astype(bf)
    selxc = make_selx().astype(bf)
    b0c = np.asarray(b0, dtype=np.float32).reshape(O, 1).copy()
    b1c = np.asarray(b1, dtype=np.float32).reshape(O, 1).copy()

    nc = _get_nc()
    in_maps = []
    for i in range(N_CORES):
        in_maps.append(
            {
                "xt": xt_pairs[i],
                "xe1h": make_xe1h(xt_all[i][0:F]),
                "a0": a0c, "w0f": w0fc, "w1g": w1gc, "selx": selxc,
                "b0": b0c, "b1": b1c,
            }
        )
    res = run_bass_kernel_spmd(nc, in_maps, core_ids=list(range(N_CORES)), trace=_trace)
    outs = []
    for r in res.results:
        o = np.concatenate([r["out0"], r["out1"]], axis=0).T
        outs.append(o)
    full = np.concatenate(outs, axis=0).astype(np.float32)
    if _trace:
        return full, res
    return full


# revision 28
# speedup vs baseline: 1.2082x; 1.2082x over previous
"""
CIN (Compressed Interaction Network) kernel for Trainium2, 8 NeuronCores.

Problem (hardcoded):
  x: [4096, 32, 64] fp32; w0: [128, 1024]; b0: [128]; w1: [128, 2048]; b1: [128]
  out: [4096, 192] = concat(relu(y0)[:, 64:], relu(y1)).sum(d)

Sharding: data parallel over batch, 512 samples/core, tokens t=(b,d), T=32768.

Key structure (per core, software-pipelined over 2048-token pairs):
  - L0 via polarization: x_h*x_f = ((x_h+x_f)^2 - x_h^2 - x_f^2)/2 ->
    496 upper-triangle sum-channels + 32 squares = 528 channels (vs 1024).
    Built on the PE as K=32 two-hot matmuls (4-way tile_position
    concurrency over the 4 x-copies); ScalarE evacuates with func=Square;
    contraction uses host-folded weights.
  - L1 z-channels are PERMUTED to (f, h) order so the broadcast operand is
    built from x (a pure input): xe_g[p, t] = x[2g + p//64, t] is
    DMA-expanded from HBM (13 groups) or PE-built via one-hot selects
    (3 groups, ScalarE evac). The hidden side is one 2x-tiled SBUF copy
    per pair (hd2[p] = hidden[p % 64]).
  - Block B interleaves: L1 muls+contract of pair B-1 with the PE s-builds
    of pair B (one build group per 2 contract groups), so the PE stream
    never has a >2us bubble (keeps HAM at K=8/8). L0 contract of pair B
    runs at block tail, producing hd2(B) before block B+1's muls need it.
  - d-sums via log2 trees of strided 2x-mode VectorE adds (not
    tensor_reduce, which is capped at 1x).
"""

import sys

import numpy as np
import ml_dtypes

sys.path.insert(0, "/opt/trn_rl_repo")

B_FULL = 4096
N_CORES = 8
BS = B_FULL // N_CORES  # 512
F = 32
D = 64
T = BS * D
PAIR = 2048
O = 128
H1 = 64
G1 = 16
CH0 = 528
NG0 = 5

DMA_G = list(range(14))   # groups delivered by plain DMA from host-expanded xe1h
PE_G = [14, 15]           # groups built on the PE from one-hot selects
GPS_MULS = (14, 15)       # z1 mul groups run on GpSimd instead of VectorE

_CACHE = {}


def _build_nc(BS=BS):
    import concourse.bass as bass
    import concourse.tile as tile
    from concourse import bacc, mybir

    T = BS * D
    NPAIR = T // PAIR
    SPP = PAIR // D

    bf16 = mybir.dt.bfloat16
    f32 = mybir.dt.float32
    Relu = mybir.ActivationFunctionType.Relu
    Square = mybir.ActivationFunctionType.Square
    Copy = mybir.ActivationFunctionType.Copy

    nc = bacc.Bacc(None, target_bir_lowering=False)

    xt = nc.dram_tensor("xt", [NPAIR, 128, PAIR], bf16, kind="ExternalInput")
    xe1h = nc.dram_tensor(
        "xe1h", [NPAIR, len(DMA_G), 128, PAIR], bf16, kind="ExternalInput"
    )
    selx = nc.dram_tensor("selx", [128, len(PE_G), 128], bf16, kind="ExternalInput")
    a0 = nc.dram_tensor("a0", [128, NG0, 128], bf16, kind="ExternalInput")
    w0f = nc.dram_tensor("w0f", [128, NG0, O], bf16, kind="ExternalInput")
    w1g = nc.dram_tensor("w1g", [128, G1, O], bf16, kind="ExternalInput")
    b0 = nc.dram_tensor("b0", [O, 1], f32, kind="ExternalInput")
    b1 = nc.dram_tensor("b1", [O, 1], f32, kind="ExternalInput")
    out0 = nc.dram_tensor("out0", [O - H1, BS], f32, kind="ExternalOutput")
    out1 = nc.dram_tensor("out1", [O, BS], f32, kind="ExternalOutput")

    with tile.TileContext(nc) as tc:
        with (
            tc.tile_pool(name="singles", bufs=1) as singles,
            tc.tile_pool(name="xrp", bufs=2) as xrp,
            tc.tile_pool(name="s0p", bufs=6) as s0p,
            tc.tile_pool(name="xep", bufs=13) as xep,
            tc.tile_pool(name="xcp", bufs=2) as xcp,
            tc.tile_pool(name="hd2p", bufs=2) as hd2p,
            tc.tile_pool(name="y0sbp", bufs=2) as y0sbp,
            tc.tile_pool(name="y1sbp", bufs=2) as y1sbp,
            tc.tile_pool(name="trp", bufs=2) as trp,
            tc.tile_pool(name="bcps", bufs=2, space="PSUM") as bcps,
            tc.tile_pool(name="yqp", bufs=4, space="PSUM") as yqp,
        ):
            a0s = singles.tile([128, NG0, 128], bf16)
            selxs = singles.tile([128, len(PE_G), 128], bf16)
            w0fs = singles.tile([128, NG0, O], bf16)
            w1s = singles.tile([128, G1, O], bf16)
            b0s = singles.tile([O, 1], f32)
            b1s = singles.tile([O, 1], f32)
            oacc0 = singles.tile([O, BS], f32)
            oacc1 = singles.tile([O, BS], f32)

            nc.gpsimd.dma_start(out=a0s[:], in_=a0[:])
            nc.gpsimd.dma_start(out=selxs[:], in_=selx[:])
            nc.gpsimd.dma_start(out=w0fs[:], in_=w0f[:])
            nc.gpsimd.dma_start(out=w1s[:], in_=w1g[:])
            nc.gpsimd.dma_start(out=b0s[:], in_=b0[:])
            nc.gpsimd.dma_start(out=b1s[:], in_=b1[:])

            st = {}

            def s1_load(P):
                """xt DMA, one block ahead (small, feeds the PE builds).
                HWDGE (sync) so it bypasses the SWDGE bulk-xe FIFO."""
                xr = xrp.tile([128, PAIR], bf16, name=f"xr{P}", tag="xr")
                nc.sync.dma_start(out=xr[:], in_=xt[P, :, :])
                st.setdefault(P, {})["xr"] = xr

            def prefetch_xe(P):
                """xe_g[p, t] = x[2g + p//64, t]; groups 0-13 via per-group
                SWDGE reads of the host-expanded xe1h, groups 14-15 PE-built."""
                xes = []
                for R in range(7):
                    xe = xep.tile([128, 2, PAIR], bf16, name=f"xe{P}_{R}", tag="xe")
                    xes.append(xe)
                xes.append(xcp.tile([128, 2, PAIR], bf16, name=f"xeC{P}", tag="xc"))
                for i, g in enumerate(DMA_G):
                    nc.gpsimd.dma_start(
                        out=xes[g // 2][:, g % 2, :], in_=xe1h[P, i, :, :]
                    )
                st[P]["xes"] = xes
                st[P]["s0"] = []

            def xeg_ap(P, g):
                return st[P]["xes"][g // 2][:, g % 2, :]

            def build_xe_pe(P, k):
                """PE-built broadcast for PE_G[k]: one-hot selects of x rows.
                Evac on ScalarE (VectorE is the mul pole)."""
                g = PE_G[k]
                xr = st[P]["xr"]
                psA = bcps.tile([128, 2, 512], f32, name=f"xpsA{P}_{k}", tag="bc")
                psB = bcps.tile([128, 2, 512], f32, name=f"xpsB{P}_{k}", tag="bc")
                for c in range(4):
                    ps = psA if c < 2 else psB
                    nc.tensor.matmul(
                        ps[:, c % 2, :],
                        selxs[32 * c : 32 * c + 32, k, :],
                        xr[32 * c : 32 * c + 32, 512 * c : 512 * (c + 1)],
                        start=True, stop=True, tile_position=(32 * c, 0),
                    )
                xeg = xeg_ap(P, g)
                for half, ps in ((0, psA), (1, psB)):
                    dst = xeg[:, 1024 * half : 1024 * (half + 1)].rearrange(
                        "p (j c) -> p j c", j=2
                    )
                    nc.scalar.activation(dst, ps[:], Copy)

            def l0_half(P, half):
                """L0 contract for tokens [1024*half, 1024*(half+1)) into a
                2-bank bcps tile; ScalarE Relu evac into y0sb; hd2 half-copy.
                Runs mid-block so hd2(P) is ready before the tail muls."""
                s0g = st[P]["s0"]
                ps = bcps.tile([128, 2, 512], f32, name=f"l0q{P}_{half}", tag="bc")
                for G in range(NG0):
                    for q in range(2):
                        nc.tensor.matmul(
                            ps[:, q, :], w0fs[:, G, :],
                            s0g[G][:, 1024 * half + 512 * q : 1024 * half + 512 * (q + 1)],
                            start=(G == 0), stop=(G == NG0 - 1),
                        )
                if half == 0:
                    y0sb = y0sbp.tile([128, PAIR], bf16, name=f"y0sb{P}", tag="y0sb")
                    st[P]["y0sb"] = y0sb
                    hd2 = hd2p.tile([128, PAIR], bf16, name=f"hd2_{P}", tag="hd2")
                    st[P]["hd2"] = hd2
                y0sb, hd2 = st[P]["y0sb"], st[P]["hd2"]
                dst = y0sb[:, 1024 * half : 1024 * (half + 1)].rearrange(
                    "p (j c) -> p j c", j=2
                )
                nc.scalar.activation(dst, ps[:], Relu, bias=b0s[:])
                sl = slice(1024 * half, 1024 * (half + 1))
                nc.sync.dma_start(out=hd2[0:64, sl], in_=y0sb[0:H1, sl])
                nc.sync.dma_start(out=hd2[64:128, sl], in_=y0sb[0:H1, sl])

            def s1_build(P, G):
                xr = st[P]["xr"]
                psA = bcps.tile([128, 2, 512], f32, name=f"psA{P}_{G}", tag="bc")
                psB = bcps.tile([128, 2, 512], f32, name=f"psB{P}_{G}", tag="bc")
                for c in range(4):
                    ps = psA if c < 2 else psB
                    nc.tensor.matmul(
                        ps[:, c % 2, :],
                        a0s[32 * c : 32 * c + 32, G, :],
                        xr[32 * c : 32 * c + 32, 512 * c : 512 * (c + 1)],
                        start=True, stop=True, tile_position=(32 * c, 0),
                    )
                sg = s0p.tile([128, PAIR], bf16, name=f"s0_{P}_{G}", tag="s0")
                for half, ps in ((0, psA), (1, psB)):
                    dst = sg[:, 1024 * half : 1024 * (half + 1)].rearrange(
                        "p (j c) -> p j c", j=2
                    )
                    nc.scalar.activation(dst, ps[:], Square)
                st[P]["s0"].append(sg)

            tr_n = [0]

            def dtree(dst, src):
                """sum over d (64, OUTER: token t = d*32 + b) via log2 tree of
                fully contiguous 2x-mode halves-adds. src: [p, 2048]; dst [p,32]."""
                p = src.shape[0]
                tr_n[0] += 1
                tr = trp.tile([128, 1024], bf16, name=f"tr{tr_n[0]}", tag="tr")
                t = tr[0:p]
                nc.vector.tensor_add(t[:, 0:1024], src[:, 0:1024], src[:, 1024:2048])
                nc.vector.tensor_add(t[:, 0:512], t[:, 0:512], t[:, 512:1024])
                nc.vector.tensor_add(t[:, 0:256], t[:, 0:256], t[:, 256:512])
                nc.vector.tensor_add(t[:, 0:128], t[:, 0:128], t[:, 128:256])
                nc.vector.tensor_add(t[:, 0:64], t[:, 0:64], t[:, 64:128])
                nc.vector.tensor_add(dst, t[:, 0:32], t[:, 32:64])

            def red0(P):
                """out0 d-sum for pair P (y0sb ready since prior block)."""
                dtree(
                    oacc0[H1:O, P * SPP : (P + 1) * SPP],
                    st[P]["y0sb"][H1:O, :],
                )

            def s3_alloc(P):
                y1sb = y1sbp.tile([128, PAIR], bf16, name=f"y1sb{P}", tag="y1sb")
                yq = [
                    yqp.tile([128, 512], f32, name=f"y1q{P}_{q}", tag="yq")
                    for q in range(4)
                ]
                st[P]["y1sb"] = y1sb
                st[P]["yq"] = yq

            def s3_mul(P, g):
                xeg, hd2 = xeg_ap(P, g), st[P]["hd2"]
                eng = nc.gpsimd if g in GPS_MULS else nc.vector
                eng.tensor_mul(xeg, xeg, hd2[:])

            def s3_contract(P, g):
                xeg, yq = xeg_ap(P, g), st[P]["yq"]
                for q in range(4):
                    nc.tensor.matmul(
                        yq[q][:], w1s[:, g, :],
                        xeg[:, 512 * q : 512 * (q + 1)],
                        start=(g == 0), stop=(g == G1 - 1),
                    )

            def s3_fini(P):
                y1sb, yq = st[P]["y1sb"], st[P]["yq"]
                for q in range(4):
                    nc.scalar.activation(
                        y1sb[:, 512 * q : 512 * (q + 1)], yq[q][:], Relu, bias=b1s[:]
                    )
                dtree(oacc1[:, P * SPP : (P + 1) * SPP], y1sb[:])
                del st[P]

            for B in range(NPAIR + 1):
                # pc: pair being multiplied + L1-contracted (muls 0,1 already
                # emitted at tail of block B-1); pn: pair being built (s0,
                # xe_pe, L0 halves, hd2) + prefetched.
                pc, pn = B - 1, B
                have_c = pc >= 0
                have_n = pn < NPAIR
                if B == 0:
                    s1_load(0)
                if have_c:
                    s3_alloc(pc)
                    for g in GPS_MULS:
                        # gpsimd: emit BEFORE the xe DMA issues so the muls
                        # aren't queued behind pool-gated dma_starts
                        s3_mul(pc, g)
                if have_n:
                    st.setdefault(pn, {})
                    if pn + 1 < NPAIR:
                        s1_load(pn + 1)
                    prefetch_xe(pn)
                # build/L0 ops of pair pn by g-slot:
                #   g=0..4 s1_build, g=5,6 xe_pe, g=9,11 L0 halves
                for g in range(G1):
                    if have_c:
                        s3_contract(pc, g)
                        if g + 2 < G1 and (g + 2) not in GPS_MULS:
                            s3_mul(pc, g + 2)
                        if g == 6:
                            red0(pc)  # vector has ~1.5us slack by here
                    if have_n:
                        if g < NG0:
                            s1_build(pn, g)
                        elif g in (5, 6):
                            build_xe_pe(pn, g - NG0)
                        elif g in (9, 11):
                            l0_half(pn, (g - 9) // 2)
                if have_n:
                    # tail muls of pair pn (hd2(pn) ready mid-block)
                    s3_mul(pn, 0)
                    s3_mul(pn, 1)
                if have_c:
                    s3_fini(pc)

            nc.gpsimd.dma_start(out=out0[:], in_=oacc0[H1:O, :])
            nc.gpsimd.dma_start(out=out1[:], in_=oacc1[:])

    nc.finalize()
    return nc


def _get_nc():
    if "nc" not in _CACHE:
        _CACHE["nc"] = _build_nc()
    return _CACHE["nc"]


def _l0_pairs():
    return [(h, f) for h in range(F) for f in range(h + 1, F)]


def make_l0(w0_np):
    """A0 build matrix [32, 640] and folded weights [640, 128] (zero-padded)."""
    pairs = _l0_pairs()
    A0 = np.zeros((F, NG0 * 128), np.float32)
    w0fold = np.zeros((NG0 * 128, O), np.float32)
    for k, (h, f) in enumerate(pairs):
        A0[h, k] = 1.0
        A0[f, k] = 1.0
        w0fold[k] = (w0_np[:, h * F + f] + w0_np[:, f * F + h]) / 2
    for h in range(F):
        k = 496 + h
        A0[h, k] = 1.0
        c = w0_np[:, h * F + h].copy()
        for f in range(F):
            if f != h:
                c -= 0.5 * (w0_np[:, h * F + f] + w0_np[:, f * F + h])
        w0fold[k] = c
    return A0, w0fold


def make_xe1h(x_core_bf):
    """Host-expanded broadcast rows, contiguous per pair:
    [NPAIR, 14, 128, PAIR]."""
    row = np.empty(len(DMA_G) * 128, np.int64)
    p = np.arange(128)
    for i, g in enumerate(DMA_G):
        row[128 * i : 128 * (i + 1)] = 2 * g + p // 64
    full = x_core_bf[row]  # [14*128, T]
    npair = full.shape[1] // PAIR
    full = full.reshape(len(DMA_G), 128, npair, PAIR)
    return np.ascontiguousarray(full.transpose(2, 0, 1, 3))


def make_selx():
    sel = np.zeros((128, len(PE_G), 128), np.float32)
    for k, g in enumerate(PE_G):
        for s in range(4):
            for m in range(128):
                sel[32 * s + 2 * g + m // 64, k, m] = 1.0
    return sel


def make_w1g(w1_np):
    """Permuted L1 weights [128, 16, 128]: channel (g, p) -> h*F + f with
    f = 2g + p//64, h = p%64."""
    w1t = np.ascontiguousarray(np.asarray(w1_np, dtype=np.float32).T)  # [2048, O]
    p = np.arange(128)
    out = np.empty((128, G1, O), np.float32)
    for g in range(G1):
        c = (p % 64) * F + 2 * g + p // 64
        out[:, g, :] = w1t[c]
    return out


def kernel(cin_inputs, w0, b0, w1, b1, _trace=False):
    from concourse.bass_utils import run_bass_kernel_spmd

    x = np.asarray(cin_inputs, dtype=np.float32)
    assert x.shape == (B_FULL, F, D)
    bf = ml_dtypes.bfloat16
    NP_ = T // PAIR
    SPP_ = PAIR // D  # 32 batches per pair
    # token layout: pair-major, then d-MAJOR within a pair (t = d*32 + b) so
    # the d-sum trees are fully contiguous halves-adds on the device
    xt_all = np.ascontiguousarray(
        x.reshape(N_CORES, NP_, SPP_, F, D).transpose(0, 3, 1, 4, 2)
    ).astype(bf).reshape(N_CORES, F, BS * D)  # [NC, F, (P, d, b)]
    xt_all = np.ascontiguousarray(np.tile(xt_all, (1, 4, 1)))  # [NC, 128, T]
    xt_pairs = np.ascontiguousarray(
        xt_all.reshape(N_CORES, 128, NP_, PAIR).transpose(0, 2, 1, 3)
    )  # [NC, NPAIR, 128, PAIR]

    w0_np = np.asarray(w0, dtype=np.float32)
    A0, w0fold = make_l0(w0_np)
    a0c = np.ascontiguousarray(
        np.tile(A0.reshape(F, NG0, 128), (4, 1, 1))
    ).astype(bf)
    w0fc = np.ascontiguousarray(
        w0fold.reshape(NG0, 128, O).transpose(1, 0, 2)
    ).astype(bf)
    w1gc = np.ascontiguousarray(make_w1g(w1)).astype(bf)
    selxc = make_selx().astype(bf)
    b0c = np.asarray(b0, dtype=np.float32).reshape(O, 1).copy()
    b1c = np.asarray(b1, dtype=np.float32).reshape(O, 1).copy()

    nc = _get_nc()
    in_maps = []
    for i in range(N_CORES):
        in_maps.append(
            {
                "xt": xt_pairs[i],
                "xe1h": make_xe1h(xt_all[i][0:F]),
                "a0": a0c, "w0f": w0fc, "w1g": w1gc, "selx": selxc,
                "b0": b0c, "b1": b1c,
            }
        )
    res = run_bass_kernel_spmd(nc, in_maps, core_ids=list(range(N_CORES)), trace=_trace)
    outs = []
    for r in res.results:
        o = np.concatenate([r["out0"], r["out1"]], axis=0).T
        outs.append(o)
    full = np.concatenate(outs, axis=0).astype(np.float32)
    if _trace:
        return full, res
    return full


# revision 29
# speedup vs baseline: 1.2769x; 1.0568x over previous
"""
CIN (Compressed Interaction Network) kernel for Trainium2, 8 NeuronCores.

Problem (hardcoded):
  x: [4096, 32, 64] fp32; w0: [128, 1024]; b0: [128]; w1: [128, 2048]; b1: [128]
  out: [4096, 192] = concat(relu(y0)[:, 64:], relu(y1)).sum(d)

Sharding: data parallel over batch, 512 samples/core, tokens t=(b,d), T=32768.

Key structure (per core, software-pipelined over 2048-token pairs):
  - L0 via polarization: x_h*x_f = ((x_h+x_f)^2 - x_h^2 - x_f^2)/2 ->
    496 upper-triangle sum-channels + 32 squares = 528 channels (vs 1024).
    Built on the PE as K=32 two-hot matmuls (4-way tile_position
    concurrency over the 4 x-copies); ScalarE evacuates with func=Square;
    contraction uses host-folded weights.
  - L1 z-channels are PERMUTED to (f, h) order so the broadcast operand is
    built from x (a pure input): xe_g[p, t] = x[2g + p//64, t] is
    DMA-expanded from HBM (13 groups) or PE-built via one-hot selects
    (3 groups, ScalarE evac). The hidden side is one 2x-tiled SBUF copy
    per pair (hd2[p] = hidden[p % 64]).
  - Block B interleaves: L1 muls+contract of pair B-1 with the PE s-builds
    of pair B (one build group per 2 contract groups), so the PE stream
    never has a >2us bubble (keeps HAM at K=8/8). L0 contract of pair B
    runs at block tail, producing hd2(B) before block B+1's muls need it.
  - d-sums via log2 trees of strided 2x-mode VectorE adds (not
    tensor_reduce, which is capped at 1x).
"""

import sys

import numpy as np
import ml_dtypes

sys.path.insert(0, "/opt/trn_rl_repo")

B_FULL = 4096
N_CORES = 8
BS = B_FULL // N_CORES  # 512
F = 32
D = 64
T = BS * D
PAIR = 2048
O = 128
H1 = 64
G1 = 16
CH0 = 528
NG0 = 5

DMA_G = list(range(14))   # groups delivered by plain DMA from host-expanded xe1h
PE_G = [14, 15]           # groups built on the PE from one-hot selects
GPS_MULS = (14, 15)       # z1 mul groups run on GpSimd instead of VectorE

_CACHE = {}


def _build_nc(BS=BS):
    import concourse.bass as bass
    import concourse.tile as tile
    from concourse import bacc, mybir

    T = BS * D
    NPAIR = T // PAIR
    SPP = PAIR // D

    bf16 = mybir.dt.bfloat16
    f32 = mybir.dt.float32
    Relu = mybir.ActivationFunctionType.Relu
    Square = mybir.ActivationFunctionType.Square
    Copy = mybir.ActivationFunctionType.Copy

    nc = bacc.Bacc(None, target_bir_lowering=False)

    xt = nc.dram_tensor("xt", [NPAIR, 128, PAIR], bf16, kind="ExternalInput")
    xe1h = nc.dram_tensor(
        "xe1h", [NPAIR, len(DMA_G), 128, PAIR], bf16, kind="ExternalInput"
    )
    selx = nc.dram_tensor("selx", [128, len(PE_G), 128], bf16, kind="ExternalInput")
    a0 = nc.dram_tensor("a0", [128, NG0, 128], bf16, kind="ExternalInput")
    w0f = nc.dram_tensor("w0f", [128, NG0, O], bf16, kind="ExternalInput")
    w1g = nc.dram_tensor("w1g", [128, G1, O], bf16, kind="ExternalInput")
    b0 = nc.dram_tensor("b0", [O, 1], f32, kind="ExternalInput")
    b1 = nc.dram_tensor("b1", [O, 1], f32, kind="ExternalInput")
    out0 = nc.dram_tensor("out0", [O - H1, BS], f32, kind="ExternalOutput")
    out1 = nc.dram_tensor("out1", [O, BS], f32, kind="ExternalOutput")

    with tile.TileContext(nc) as tc:
        with (
            tc.tile_pool(name="singles", bufs=1) as singles,
            tc.tile_pool(name="xrp", bufs=2) as xrp,
            tc.tile_pool(name="s0p", bufs=6) as s0p,
            tc.tile_pool(name="xep", bufs=13) as xep,
            tc.tile_pool(name="xcp", bufs=2) as xcp,
            tc.tile_pool(name="hd2p", bufs=2) as hd2p,
            tc.tile_pool(name="y0sbp", bufs=2) as y0sbp,
            tc.tile_pool(name="y1sbp", bufs=2) as y1sbp,
            tc.tile_pool(name="trp", bufs=2) as trp,
            tc.tile_pool(name="bcps", bufs=2, space="PSUM") as bcps,
            tc.tile_pool(name="yqp", bufs=4, space="PSUM") as yqp,
        ):
            a0s = singles.tile([128, NG0, 128], bf16)
            selxs = singles.tile([128, len(PE_G), 128], bf16)
            w0fs = singles.tile([128, NG0, O], bf16)
            w1s = singles.tile([128, G1, O], bf16)
            b0s = singles.tile([O, 1], f32)
            b1s = singles.tile([O, 1], f32)
            oacc0 = singles.tile([O, BS], f32)
            oacc1 = singles.tile([O, BS], f32)

            nc.gpsimd.dma_start(out=a0s[:], in_=a0[:])
            nc.gpsimd.dma_start(out=selxs[:], in_=selx[:])
            nc.gpsimd.dma_start(out=w0fs[:], in_=w0f[:])
            nc.gpsimd.dma_start(out=w1s[:], in_=w1g[:])
            nc.gpsimd.dma_start(out=b0s[:], in_=b0[:])
            nc.gpsimd.dma_start(out=b1s[:], in_=b1[:])

            st = {}

            def s1_load(P):
                """xt DMA, one block ahead (small, feeds the PE builds).
                HWDGE (sync) so it bypasses the SWDGE bulk-xe FIFO."""
                xr = xrp.tile([128, PAIR], bf16, name=f"xr{P}", tag="xr")
                nc.sync.dma_start(out=xr[:], in_=xt[P, :, :])
                st.setdefault(P, {})["xr"] = xr

            def prefetch_xe(P):
                """xe_g[p, t] = x[2g + p//64, t]; groups 0-13 via per-group
                SWDGE reads of the host-expanded xe1h, groups 14-15 PE-built."""
                xes = []
                for R in range(7):
                    xe = xep.tile([128, 2, PAIR], bf16, name=f"xe{P}_{R}", tag="xe")
                    xes.append(xe)
                xes.append(xcp.tile([128, 2, PAIR], bf16, name=f"xeC{P}", tag="xc"))
                for i, g in enumerate(DMA_G):
                    nc.gpsimd.dma_start(
                        out=xes[g // 2][:, g % 2, :], in_=xe1h[P, i, :, :]
                    )
                st[P]["xes"] = xes
                st[P]["s0"] = []

            def xeg_ap(P, g):
                return st[P]["xes"][g // 2][:, g % 2, :]

            def build_xe_pe(P, k):
                """PE-built broadcast for PE_G[k]: one-hot selects of x rows.
                Evac on ScalarE (VectorE is the mul pole)."""
                g = PE_G[k]
                xr = st[P]["xr"]
                psA = bcps.tile([128, 2, 512], f32, name=f"xpsA{P}_{k}", tag="bc")
                psB = bcps.tile([128, 2, 512], f32, name=f"xpsB{P}_{k}", tag="bc")
                for c in range(4):
                    ps = psA if c < 2 else psB
                    nc.tensor.matmul(
                        ps[:, c % 2, :],
                        selxs[32 * c : 32 * c + 32, k, :],
                        xr[32 * c : 32 * c + 32, 512 * c : 512 * (c + 1)],
                        start=True, stop=True, tile_position=(32 * c, 0),
                    )
                xeg = xeg_ap(P, g)
                for half, ps in ((0, psA), (1, psB)):
                    dst = xeg[:, 1024 * half : 1024 * (half + 1)].rearrange(
                        "p (j c) -> p j c", j=2
                    )
                    nc.scalar.activation(dst, ps[:], Copy)

            def l0_half(P, half):
                """L0 contract for tokens [1024*half, 1024*(half+1)) into a
                2-bank bcps tile; ScalarE Relu evac into y0sb; hd2 half-copy.
                Runs mid-block so hd2(P) is ready before the tail muls."""
                s0g = st[P]["s0"]
                ps = bcps.tile([128, 2, 512], f32, name=f"l0q{P}_{half}", tag="bc")
                for G in range(NG0):
                    for q in range(2):
                        nc.tensor.matmul(
                            ps[:, q, :], w0fs[:, G, :],
                            s0g[G][:, 1024 * half + 512 * q : 1024 * half + 512 * (q + 1)],
                            start=(G == 0), stop=(G == NG0 - 1),
                        )
                if half == 0:
                    y0sb = y0sbp.tile([128, PAIR], bf16, name=f"y0sb{P}", tag="y0sb")
                    st[P]["y0sb"] = y0sb
                    hd2 = hd2p.tile([128, PAIR], bf16, name=f"hd2_{P}", tag="hd2")
                    st[P]["hd2"] = hd2
                y0sb, hd2 = st[P]["y0sb"], st[P]["hd2"]
                dst = y0sb[:, 1024 * half : 1024 * (half + 1)].rearrange(
                    "p (j c) -> p j c", j=2
                )
                nc.scalar.activation(dst, ps[:], Relu, bias=b0s[:])
                sl = slice(1024 * half, 1024 * (half + 1))
                nc.sync.dma_start(out=hd2[0:64, sl], in_=y0sb[0:H1, sl])
                nc.sync.dma_start(out=hd2[64:128, sl], in_=y0sb[0:H1, sl])

            def s1_build(P, G):
                xr = st[P]["xr"]
                psA = bcps.tile([128, 2, 512], f32, name=f"psA{P}_{G}", tag="bc")
                psB = bcps.tile([128, 2, 512], f32, name=f"psB{P}_{G}", tag="bc")
                for c in range(4):
                    ps = psA if c < 2 else psB
                    nc.tensor.matmul(
                        ps[:, c % 2, :],
                        a0s[32 * c : 32 * c + 32, G, :],
                        xr[32 * c : 32 * c + 32, 512 * c : 512 * (c + 1)],
                        start=True, stop=True, tile_position=(32 * c, 0),
                    )
                sg = s0p.tile([128, PAIR], bf16, name=f"s0_{P}_{G}", tag="s0")
                for half, ps in ((0, psA), (1, psB)):
                    dst = sg[:, 1024 * half : 1024 * (half + 1)].rearrange(
                        "p (j c) -> p j c", j=2
                    )
                    nc.scalar.activation(dst, ps[:], Square)
                st[P]["s0"].append(sg)

            tr_n = [0]

            def dtree(dst, src):
                """sum over d (64, OUTER: token t = d*32 + b) via log2 tree of
                fully contiguous 2x-mode halves-adds. src: [p, 2048]; dst [p,32]."""
                p = src.shape[0]
                tr_n[0] += 1
                tr = trp.tile([128, 1024], bf16, name=f"tr{tr_n[0]}", tag="tr")
                t = tr[0:p]
                nc.vector.tensor_add(t[:, 0:1024], src[:, 0:1024], src[:, 1024:2048])
                nc.vector.tensor_add(t[:, 0:512], t[:, 0:512], t[:, 512:1024])
                nc.vector.tensor_add(t[:, 0:256], t[:, 0:256], t[:, 256:512])
                nc.vector.tensor_add(t[:, 0:128], t[:, 0:128], t[:, 128:256])
                nc.vector.tensor_add(t[:, 0:64], t[:, 0:64], t[:, 64:128])
                nc.vector.tensor_add(dst, t[:, 0:32], t[:, 32:64])

            def red0(P):
                """out0 d-sum for pair P (y0sb ready since prior block)."""
                dtree(
                    oacc0[H1:O, P * SPP : (P + 1) * SPP],
                    st[P]["y0sb"][H1:O, :],
                )

            def s3_alloc(P):
                y1sb = y1sbp.tile([128, PAIR], bf16, name=f"y1sb{P}", tag="y1sb")
                yq = [
                    yqp.tile([128, 512], f32, name=f"y1q{P}_{q}", tag="yq")
                    for q in range(4)
                ]
                st[P]["y1sb"] = y1sb
                st[P]["yq"] = yq

            def s3_mul(P, g):
                xeg, hd2 = xeg_ap(P, g), st[P]["hd2"]
                eng = nc.gpsimd if g in GPS_MULS else nc.vector
                eng.tensor_mul(xeg, xeg, hd2[:])

            def s3_contract(P, g):
                xeg, yq = xeg_ap(P, g), st[P]["yq"]
                for q in range(4):
                    nc.tensor.matmul(
                        yq[q][:], w1s[:, g, :],
                        xeg[:, 512 * q : 512 * (q + 1)],
                        start=(g == 0), stop=(g == G1 - 1),
                    )

            def s3_fini(P):
                y1sb, yq = st[P]["y1sb"], st[P]["yq"]
                for q in range(4):
                    nc.scalar.activation(
                        y1sb[:, 512 * q : 512 * (q + 1)], yq[q][:], Relu, bias=b1s[:]
                    )
                dtree(oacc1[:, P * SPP : (P + 1) * SPP], y1sb[:])
                del st[P]

            for B in range(NPAIR + 1):
                # pc: pair being multiplied + L1-contracted (muls 0,1 already
                # emitted at tail of block B-1); pn: pair being built (s0,
                # xe_pe, L0 halves, hd2) + prefetched.
                pc, pn = B - 1, B
                have_c = pc >= 0
                have_n = pn < NPAIR
                if B == 0:
                    s1_load(0)
                if have_c:
                    s3_alloc(pc)
                    for g in GPS_MULS:
                        # gpsimd: emit BEFORE the xe DMA issues so the muls
                        # aren't queued behind pool-gated dma_starts
                        s3_mul(pc, g)
                if have_n:
                    st.setdefault(pn, {})
                    if pn + 1 < NPAIR:
                        s1_load(pn + 1)
                    prefetch_xe(pn)
                # build/L0 ops of pair pn by g-slot:
                #   g=0..4 s1_build, g=5,6 xe_pe, g=9,11 L0 halves
                for g in range(G1):
                    if have_c:
                        s3_contract(pc, g)
                        if g + 2 < G1 and (g + 2) not in GPS_MULS:
                            s3_mul(pc, g + 2)
                        if g == 6:
                            red0(pc)  # vector has ~1.5us slack by here
                    if have_n:
                        if g < NG0:
                            s1_build(pn, g)
                        elif g in (8, 10):
                            # L0 halves early: hd2(pn) must be ready before
                            # the tail muls (chain: relu evac -> hd2 DMA)
                            l0_half(pn, (g - 8) // 2)
                        elif g in (12, 13):
                            # xe_pe late: its consumers (gpsimd muls of pn)
                            # only run at the start of the next block
                            build_xe_pe(pn, g - 12)
                if have_n:
                    # tail muls of pair pn (hd2(pn) ready mid-block)
                    s3_mul(pn, 0)
                    s3_mul(pn, 1)
                if have_c:
                    s3_fini(pc)

            nc.gpsimd.dma_start(out=out0[:], in_=oacc0[H1:O, :])
            nc.gpsimd.dma_start(out=out1[:], in_=oacc1[:])

    nc.finalize()
    return nc


def _get_nc():
    if "nc" not in _CACHE:
        _CACHE["nc"] = _build_nc()
    return _CACHE["nc"]


def _l0_pairs():
    return [(h, f) for h in range(F) for f in range(h + 1, F)]


def make_l0(w0_np):
    """A0 build matrix [32, 640] and folded weights [640, 128] (zero-padded)."""
    pairs = _l0_pairs()
    A0 = np.zeros((F, NG0 * 128), np.float32)
    w0fold = np.zeros((NG0 * 128, O), np.float32)
    for k, (h, f) in enumerate(pairs):
        A0[h, k] = 1.0
        A0[f, k] = 1.0
        w0fold[k] = (w0_np[:, h * F + f] + w0_np[:, f * F + h]) / 2
    for h in range(F):
        k = 496 + h
        A0[h, k] = 1.0
        c = w0_np[:, h * F + h].copy()
        for f in range(F):
            if f != h:
                c -= 0.5 * (w0_np[:, h * F + f] + w0_np[:, f * F + h])
        w0fold[k] = c
    return A0, w0fold


def make_xe1h(x_core_bf):
    """Host-expanded broadcast rows, contiguous per pair:
    [NPAIR, 14, 128, PAIR]."""
    row = np.empty(len(DMA_G) * 128, np.int64)
    p = np.arange(128)
    for i, g in enumerate(DMA_G):
        row[128 * i : 128 * (i + 1)] = 2 * g + p // 64
    full = x_core_bf[row]  # [14*128, T]
    npair = full.shape[1] // PAIR
    full = full.reshape(len(DMA_G), 128, npair, PAIR)
    return np.ascontiguousarray(full.transpose(2, 0, 1, 3))


def make_selx():
    sel = np.zeros((128, len(PE_G), 128), np.float32)
    for k, g in enumerate(PE_G):
        for s in range(4):
            for m in range(128):
                sel[32 * s + 2 * g + m // 64, k, m] = 1.0
    return sel


def make_w1g(w1_np):
    """Permuted L1 weights [128, 16, 128]: channel (g, p) -> h*F + f with
    f = 2g + p//64, h = p%64."""
    w1t = np.ascontiguousarray(np.asarray(w1_np, dtype=np.float32).T)  # [2048, O]
    p = np.arange(128)
    out = np.empty((128, G1, O), np.float32)
    for g in range(G1):
        c = (p % 64) * F + 2 * g + p // 64
        out[:, g, :] = w1t[c]
    return out


def kernel(cin_inputs, w0, b0, w1, b1, _trace=False):
    from concourse.bass_utils import run_bass_kernel_spmd

    x = np.asarray(cin_inputs, dtype=np.float32)
    assert x.shape == (B_FULL, F, D)
    bf = ml_dtypes.bfloat16
    NP_ = T // PAIR
    SPP_ = PAIR // D  # 32 batches per pair
    # token layout: pair-major, then d-MAJOR within a pair (t = d*32 + b) so
    # the d-sum trees are fully contiguous halves-adds on the device
    xt_all = np.ascontiguousarray(
        x.reshape(N_CORES, NP_, SPP_, F, D).transpose(0, 3, 1, 4, 2)
    ).astype(bf).reshape(N_CORES, F, BS * D)  # [NC, F, (P, d, b)]
    xt_all = np.ascontiguousarray(np.tile(xt_all, (1, 4, 1)))  # [NC, 128, T]
    xt_pairs = np.ascontiguousarray(
        xt_all.reshape(N_CORES, 128, NP_, PAIR).transpose(0, 2, 1, 3)
    )  # [NC, NPAIR, 128, PAIR]

    w0_np = np.asarray(w0, dtype=np.float32)
    A0, w0fold = make_l0(w0_np)
    a0c = np.ascontiguousarray(
        np.tile(A0.reshape(F, NG0, 128), (4, 1, 1))
    ).astype(bf)
    w0fc = np.ascontiguousarray(
        w0fold.reshape(NG0, 128, O).transpose(1, 0, 2)
    ).astype(bf)
    w1gc = np.ascontiguousarray(make_w1g(w1)).astype(bf)
    selxc = make_selx().astype(bf)
    b0c = np.asarray(b0, dtype=np.float32).reshape(O, 1).copy()
    b1c = np.asarray(b1, dtype=np.float32).reshape(O, 1).copy()

    nc = _get_nc()
    in_maps = []
    for i in range(N_CORES):
        in_maps.append(
            {
                "xt": xt_pairs[i],
                "xe1h": make_xe1h(xt_all[i][0:F]),
                "a0": a0c, "w0f": w0fc, "w1g": w1gc, "selx": selxc,
                "b0": b0c, "b1": b1c,
            }
        )
    res = run_bass_kernel_spmd(nc, in_maps, core_ids=list(range(N_CORES)), trace=_trace)
    outs = []
    for r in res.results:
        o = np.concatenate([r["out0"], r["out1"]], axis=0).T
        outs.append(o)
    full = np.concatenate(outs, axis=0).astype(np.float32)
    if _trace:
        return full, res
    return full
